# revision 24
# baseline (speedup 1.0000x reference)
"""Trainium2 Bass kernel for nn_Pndb_43344809951805 (scatter_memory).

Data-parallel over batch B=16 across 8 NeuronCores (2 batches/core).

Key algebraic restructure: scores = q@(raw@Wk.T).T = (q@Wk)@raw.T, so the
[D,D] k/ko GEMMs are replaced by tiny host-side [Q,D] projections (qWk, qWo).
The v-gate (raw.Wi) and u-gate (pd.Wu1) dot products ride along as a 65th
output row of the scores/s2 matmuls with sign-flipped weights, so one Exp
activation yields both exp(scores) and exp(-gate_arg). Phase 1 recovers the
gate row via the 65-wide PE transposes; phase 2 recovers it via a third
column of the Z2/G2 ones|aw matmul. All big matmuls are fp8e4 DoubleRow
(K=256/instruction). A = mean over B needs an AllReduce of the per-core
partial [Q,D] A (plus the A.Wu2 row), overlapped with phase-2 partA work.
"""
import sys

sys.path.insert(0, "/opt/trn_rl_repo")

import numpy as np
import ml_dtypes

import concourse.bass as bass
import concourse.bacc as bacc
import concourse.mybir as mybir
import concourse.tile as tile
from concourse import masks
from concourse.bass_utils import run_bass_kernel_spmd

F32 = mybir.dt.float32
BF16 = mybir.dt.bfloat16
F8 = mybir.dt.float8e4
AF = mybir.ActivationFunctionType
ALU = mybir.AluOpType
DR = mybir.MatmulPerfMode.DoubleRow
BF = ml_dtypes.bfloat16
F8NP = mybir.dt.np(mybir.dt.float8e4)
RAW_SC = 16.0    # fp8 scale on raw/post_dec activations
QW_SC = 16.0     # fp8 scale on qWk/qWo/-Wi/-Wu1 stationary weights
SC_SC = RAW_SC * QW_SC          # psum score scale (256)
INV_SC = 1.0 / SC_SC

B, S, D, Q = 16, 2048, 1024, 64
Q1 = Q + 1                # extra gate row
NCORES = 8
BL = B // NCORES          # local batches per core
SBLK = 512                # s-block (matmul moving free dim)
NSB = S // SBLK           # 4 s-blocks per batch
NCH = S // 128            # 16 s-chunks per batch
NJ = D // 128             # 8 contraction chunks
CPB = SBLK // 128         # 4 chunks per s-block

_prog_cache = {}


def _build(stage: str = "full"):
    nc = bacc.Bacc("TRN2", target_bir_lowering=False, debug=False,
                   enable_asserts=False, num_devices=NCORES)

    rawT_d = nc.dram_tensor("rawT", [BL, D, S], F8, kind="ExternalInput")
    rawN_d = nc.dram_tensor("rawN", [BL, S, D], F8, kind="ExternalInput")
    pdT_d = nc.dram_tensor("pdT", [BL, D, S], F8, kind="ExternalInput")
    pdN_d = nc.dram_tensor("pdN", [BL, S, D], BF16, kind="ExternalInput")
    qwk_d = nc.dram_tensor("qwk", [D, Q1], F8, kind="ExternalInput")
    qwo_d = nc.dram_tensor("qwo", [D, Q1], F8, kind="ExternalInput")
    bq1_d = nc.dram_tensor("bq1", [Q1, 1], F32, kind="ExternalInput")
    bq2_d = nc.dram_tensor("bq2", [Q1, 1], F32, kind="ExternalInput")
    wu2B_d = nc.dram_tensor("wu2B", [Q, D], F32, kind="ExternalInput")
    out_d = nc.dram_tensor("out", [BL, S, D], F32, kind="ExternalOutput")

    # [D, X] -> [128, NJ, X] chunked view for single-descriptor DMA
    def chunked(ap):
        return ap.rearrange("(j p) x -> p j x", p=128)

    with tile.TileContext(nc) as tc:
        with (
            tc.tile_pool(name="const", bufs=1) as cp,
            tc.tile_pool(name="dram", bufs=1, space="DRAM") as dram,
        ):
            ident = cp.tile([128, 128], BF16, tag="ident")
            masks.make_identity(nc, ident[:])

            # per-chunk width padded to 80: DoubleRow LDWEIGHTS requires the
            # inter-subtile step to be a multiple of 16
            QP = 80
            qwk8 = cp.tile([128, NJ, QP], F8, tag="qwk8")
            qwo8 = cp.tile([128, NJ, QP], F8, tag="qwo8")
            bq1 = cp.tile([Q1, 1], F32, tag="bq1")
            bq2 = cp.tile([Q1, 1], F32, tag="bq2")
            wu2B = cp.tile([Q, D], F32, tag="wu2B")

            # critical-path weights first
            nc.sync.dma_start(qwk8[:, :, 0:Q1], chunked(qwk_d[:, :]))
            nc.sync.dma_start(bq1[:], bq1_d[:, :])

            A_acc = cp.tile([Q, D], F32, tag="A_acc")
            A_f32 = cp.tile([Q, D], F32, tag="A_f32")
            A_bf = cp.tile([Q, D], BF16, tag="A_bf")
            # zg rhs: [ones | aw | e_gate] ([Q1, 3]); aw lands after the AR
            awo = cp.tile([Q1, 3], BF16, tag="awo")
            nc.vector.memset(awo[:], 0.0)
            nc.vector.memset(awo[0:Q, 0:1], 1.0)
            nc.vector.memset(awo[Q:Q1, 2:3], 1.0)
            scrA = cp.tile([Q, D], F32, tag="scrA")
            aw = cp.tile([Q, 1], F32, tag="aw")
            ar_in = dram.tile([Q + 1, D + 8], BF16)
            ar_out = dram.tile([Q + 1, D + 8], BF16)
            awz = cp.tile([Q, 8], BF16, tag="awz")
            nc.vector.memset(awz[:], 0.0)
            zrow = cp.tile([1, D + 8], BF16, tag="zrow")
            nc.vector.memset(zrow[:], 0.0)
            nc.gpsimd.dma_start(ar_in[0:Q, D:D + 8], awz[:])
            nc.gpsimd.dma_start(ar_in[Q:Q + 1, Q:D + 8], zrow[:, Q:D + 8])

            # ================= PHASE 1 =================
            with (
                tc.tile_pool(name="p1", bufs=1) as p1,
                tc.tile_pool(name="p1ps", bufs=1, space="PSUM") as p1ps,
            ):
                PF = 3  # prefetch depth (sblocks ahead)

                def load_raw8(b, sb):
                    t = p1.tile([128, NJ, SBLK], F8, tag="raw8",
                                name="raw8", bufs=PF + 1)
                    nc.sync.dma_start(
                        t[:], chunked(rawT_d[b])[
                            :, :, sb * SBLK:(sb + 1) * SBLK])
                    return t

                def load_rns(b, sb):
                    t = p1.tile([128, CPB, D], F8, tag="rns",
                                name="rns", bufs=PF + 1)
                    nc.sync.dma_start(
                        t[:],
                        rawN_d[b, sb * SBLK:(sb + 1) * SBLK, :].rearrange(
                            "(c p) d -> p c d", p=128))
                    return t

                all_p1 = [(b, sb) for b in range(BL) for sb in range(NSB)]
                fifo = [(load_raw8(*all_p1[k]), load_rns(*all_p1[k]))
                        for k in range(PF)]
                nc.sync.dma_start(qwo8[:, :, 0:Q1], chunked(qwo_d[:, :]))
                nc.sync.dma_start(bq2[:], bq2_d[:, :])
                nc.sync.dma_start(wu2B[:], wu2B_d[:])

                # software pipeline: front(i) = scores+exp, back(i-1) =
                # transposes+gate+A-matmuls. Keeps the tensor queue free of
                # head-of-line waits on the scalar/vector gate chain.
                zp_of = {}
                aps_of = {}
                state = None

                def front(idx):
                    b, sb = all_p1[idx]
                    raw8_cur, rns_cur = fifo.pop(0)
                    if idx + PF < len(all_p1):
                        fifo.append((load_raw8(*all_p1[idx + PF]),
                                     load_rns(*all_p1[idx + PF])))
                    if sb == 0:
                        zp_of[b] = p1.tile([Q1, NSB], F32, tag="Zp", bufs=2,
                                           name="Zp")
                    # scores (+gate row 64) = qwk8.T @ raw8, fp8 DR
                    sc_ps = p1ps.tile([Q1, SBLK], F32, tag="sc_ps", bufs=2)
                    for j0 in range(NJ // 2):
                        nc.tensor.matmul(
                            sc_ps[:],
                            qwk8[:, 2 * j0:2 * j0 + 2, 0:Q1],
                            raw8_cur[:, 2 * j0:2 * j0 + 2, :],
                            start=(j0 == 0), stop=(j0 == NJ // 2 - 1),
                            perf_mode=DR)
                    # U rows 0..63 = exp(scores); row 64 = exp(-raw.Wi-bi)
                    U = p1.tile([Q1, SBLK], BF16, tag="U", bufs=2)
                    nc.scalar.activation(U[:], sc_ps[:], AF.Exp,
                                         scale=INV_SC, bias=bq1[:],
                                         accum_out=zp_of[b][:, sb:sb + 1])
                    return (U, rns_cur, b, sb)

                def back(st):
                    U, rns_cur, b, sb = st
                    if sb == 0:
                        aps_of[b] = p1ps.tile([Q, D], F32, tag="A_ps",
                                              bufs=2, name="A_ps")
                    A_ps = aps_of[b]
                    # transpose (65-wide: gate-exp rides in column 64);
                    # per-chunk width padded to 66 so each bf16 slice stays
                    # 4-byte aligned in PSUM
                    ut_ps = p1ps.tile([128, CPB, Q1 + 1], BF16,
                                      tag="ut_ps", bufs=2)
                    for cc in range(CPB):
                        nc.tensor.transpose(
                            ut_ps[:, cc, 0:Q1],
                            U[:, cc * 128:(cc + 1) * 128],
                            ident[:Q1, :Q1])
                    # Gg = sigmoid(raw.Wi+bi) = 1/(1+exp(-)) per chunk
                    Gg = p1.tile([128, CPB], F32, tag="Gg", bufs=2)
                    nc.vector.tensor_scalar_add(
                        Gg[:], ut_ps[:, :, Q:Q + 1].squeeze(), 1.0)
                    nc.vector.reciprocal(Gg[:], Gg[:])
                    uts = p1.tile([128, CPB, Q], F8, tag="uts", bufs=2)
                    for cc in range(CPB):
                        nc.scalar.activation(uts[:, cc, :],
                                             ut_ps[:, cc, 0:Q], AF.Copy,
                                             scale=Gg[:, cc:cc + 1])
                    # A += (U*g).T @ rawN, fp8 DR over chunk pairs
                    for c0 in range(CPB // 2):
                        cp2 = sb * (CPB // 2) + c0
                        for h in range(2):
                            hsl = slice(h * 512, (h + 1) * 512)
                            nc.tensor.matmul(
                                A_ps[:, hsl],
                                uts[:, 2 * c0:2 * c0 + 2, :],
                                rns_cur[:, 2 * c0:2 * c0 + 2, hsl],
                                start=(cp2 == 0),
                                stop=(cp2 == NSB * CPB // 2 - 1),
                                perf_mode=DR, skip_group_check=True)
                    if sb == NSB - 1:
                        # A_acc += A_ps / (16 * B * Z)  (16 = fp8 rawN scale)
                        Z1 = p1.tile([Q, 1], F32, tag="Z1", bufs=2)
                        nc.vector.tensor_reduce(Z1[:], zp_of[b][0:Q, :],
                                                mybir.AxisListType.X,
                                                ALU.add)
                        sA = p1.tile([Q, 1], F32, tag="sA", bufs=2)
                        nc.vector.reciprocal(sA[:], Z1[:])
                        nc.vector.tensor_scalar_mul(sA[:], sA[:],
                                                    1.0 / (B * RAW_SC))
                        if b == 0:
                            nc.vector.tensor_scalar_mul(A_acc[:], A_ps[:],
                                                        sA[:])
                        else:
                            nc.vector.scalar_tensor_tensor(
                                A_acc[:], A_ps[:], sA[:], A_acc[:],
                                ALU.mult, ALU.add)

                for idx in range(len(all_p1)):
                    st = front(idx)
                    if state is not None:
                        back(state)
                    state = st
                back(state)

                # aw_local = A_acc . Wu2 rides along in the AllReduce
                nc.vector.scalar_tensor_tensor(
                    scrA[:], A_acc[:], 1.0, wu2B[:],
                    ALU.mult, ALU.mult, accum_out=aw[:])
                nc.gpsimd.dma_start(ar_in[0:Q, 0:D], A_acc[:])
                nc.gpsimd.dma_start(
                    ar_in[Q:Q + 1, 0:Q].rearrange("a b -> b a"), aw[:])

            # ---- AllReduce of partial A across the 8 cores ----
            if stage == "p1":
                nc.sync.dma_start(out_d[0, 0:Q, :], A_acc[:])
            else:
                if stage == "p2":
                    arr = ar_in
                else:
                    nc.gpsimd.collective_compute(
                        "AllReduce", ALU.add,
                        replica_groups=[list(range(NCORES))],
                        ins=[ar_in.opt()], outs=[ar_out.opt()],
                    )
                    arr = ar_out
                if stage == "ar":
                    nc.gpsimd.dma_start(A_f32[:], arr[0:Q, 0:D])
                nc.gpsimd.dma_start(A_bf[:], arr[0:Q, 0:D])
                nc.gpsimd.dma_start(
                    awo[0:Q, 1:2], arr[Q:Q + 1, 0:Q].rearrange("a b -> b a"))

            # ================= PHASE 2 =================
            if stage == "p1":
                pass  # skip phase 2
            else:
              with (
                  tc.tile_pool(name="p2", bufs=1) as p2,
                  tc.tile_pool(name="p2ps", bufs=1, space="PSUM") as p2ps,
              ):
                  NP2 = BL * NSB + 1

                  def part_a(b, sb):
                      ssl = slice(sb * SBLK, (sb + 1) * SBLK)
                      pdt8 = p2.tile([128, NJ, SBLK], F8, tag="pdt8",
                                     name="pdt8", bufs=2)
                      nc.sync.dma_start(pdt8[:], chunked(pdT_d[b])[:, :, ssl])
                      pdn = p2.tile([128, CPB, D], BF16, tag="pdn",
                                    name="pdn", bufs=NP2)
                      nc.sync.dma_start(
                          pdn[:],
                          pdN_d[b, ssl, :].rearrange("(c p) d -> p c d",
                                                     p=128))
                      # s2 (+gate row 64) = qwo8.T @ pdt8, fp8 DR
                      sc2_ps = p2ps.tile([Q1, SBLK], F32, tag="s2_ps",
                                         bufs=2)
                      for j0 in range(NJ // 2):
                          nc.tensor.matmul(
                              sc2_ps[:],
                              qwo8[:, 2 * j0:2 * j0 + 2, 0:Q1],
                              pdt8[:, 2 * j0:2 * j0 + 2, :],
                              start=(j0 == 0), stop=(j0 == NJ // 2 - 1),
                              perf_mode=DR)
                      # U2 rows 0..63 = exp(s2); row 64 = exp(-pd.Wu1-cg)
                      U2 = p2.tile([Q1, SBLK], BF16, tag="U2", bufs=NP2)
                      nc.scalar.activation(U2[:], sc2_ps[:], AF.Exp,
                                           scale=INV_SC, bias=bq2[:])
                      return (U2, pdn, b, sb)

                  def part_b_z(st):
                      (U2, pdn, b, sb) = st
                      # per chunk: [Z2 | G2 | E1] = U2[q,s].T @ [ones|aw|e64]
                      zg_ps = p2ps.tile([128, CPB, 3], F32, tag="zg",
                                        bufs=2)
                      for cc in range(CPB):
                          nc.tensor.matmul(
                              zg_ps[:, cc, :],
                              U2[:, cc * 128:(cc + 1) * 128], awo[:],
                              start=(cc == 0), stop=(cc == CPB - 1),
                              skip_group_check=True)
                      # SC = sigmoid(G1+G2/Z2+cg)/Z2
                      #    = 1/((1+E1*exp(-G2/Z2))*Z2)
                      rz = p2.tile([128, CPB], F32, tag="rz", bufs=2)
                      nc.vector.reciprocal(rz[:],
                                           zg_ps[:, :, 0:1].squeeze())
                      t4 = p2.tile([128, CPB], F32, tag="t4", bufs=2)
                      nc.vector.tensor_mul(t4[:], zg_ps[:, :, 1:2].squeeze(),
                                           rz[:])
                      e4 = p2.tile([128, CPB], F32, tag="e4", bufs=2)
                      nc.scalar.activation(e4[:], t4[:], AF.Exp, scale=-1.0)
                      nc.vector.tensor_mul(e4[:], e4[:],
                                           zg_ps[:, :, 2:3].squeeze())
                      nc.vector.tensor_scalar_add(e4[:], e4[:], 1.0)
                      nc.vector.reciprocal(e4[:], e4[:])
                      SC = p2.tile([128, CPB], F32, tag="SC", bufs=2)
                      nc.vector.tensor_mul(SC[:], e4[:], rz[:])
                      return (U2, pdn, SC, b, sb)

                  def part_b_out(st):
                      (U2, pdn, SC, b, sb) = st
                      # A2 + fused residual; h0 on DVE, h1 via scalar+gpsimd
                      for cc in range(CPB):
                          c = sb * CPB + cc
                          outt = p2.tile([128, D], F32, tag="outt", bufs=3)
                          for h in range(2):
                              hsl = slice(h * 512, (h + 1) * 512)
                              a2_ps = p2ps.tile([128, 512], F32,
                                                tag="a2_ps", bufs=4)
                              nc.tensor.matmul(
                                  a2_ps[:], U2[0:Q, cc * 128:(cc + 1) * 128],
                                  A_bf[:, hsl], start=True, stop=True)
                              if h == 0:
                                  nc.vector.scalar_tensor_tensor(
                                      outt[:, hsl], a2_ps[:],
                                      SC[:, cc:cc + 1],
                                      pdn[:, cc, hsl], ALU.mult, ALU.add)
                              else:
                                  a2s = p2.tile([128, 512], F32, tag="a2s",
                                                bufs=2)
                                  nc.scalar.activation(
                                      a2s[:], a2_ps[:], AF.Copy,
                                      scale=SC[:, cc:cc + 1])
                                  nc.gpsimd.tensor_tensor(
                                      outt[:, hsl], a2s[:],
                                      pdn[:, cc, hsl], ALU.add)
                          nc.gpsimd.dma_start(
                              out_d[b, c * 128:(c + 1) * 128, :], outt[:])

                  all_sb = [(b, sb) for b in range(BL) for sb in range(NSB)]
                  pending = [part_a(b, sb) for b, sb in all_sb]
                  zstates = []
                  for st in pending:
                      z = part_b_z(st)
                      if zstates:
                          part_b_out(zstates.pop(0))
                      zstates.append(z)
                  for z in zstates:
                      part_b_out(z)

            if stage == "ar":
                nc.sync.dma_start(out_d[0, 0:Q, :], A_f32[:])
    nc.compile()
    return nc


def _get_prog():
    if "p" not in _prog_cache:
        _prog_cache["p"] = _build()
    return _prog_cache["p"]


def kernel(raw, post_dec, mask, questions, Wk, bk, Wi, bi, Wo, bo,
           Wu1, bu1, Wu2, bu2, b1, _trace=False):
    raw = np.asarray(raw, dtype=np.float32)
    post_dec = np.asarray(post_dec, dtype=np.float32)
    questions = np.asarray(questions, dtype=np.float32)
    Wk = np.asarray(Wk, dtype=np.float32)
    Wo = np.asarray(Wo, dtype=np.float32)

    bi_v = float(np.asarray(bi).reshape(-1)[0])
    cgate_v = float(np.asarray(bu1).reshape(-1)[0]
                    + np.asarray(bu2).reshape(-1)[0]
                    + np.asarray(b1).reshape(-1)[0])
    nc = _get_prog()

    inv_sqrt_d = np.float32(1.0 / np.sqrt(D))
    inv_sqrt_q = np.float32(1.0 / np.sqrt(Q))

    def to_f8(x):
        return np.clip(x, -240.0, 240.0).astype(F8NP)

    bk_v = np.asarray(bk, np.float32).reshape(D)
    bo_v = np.asarray(bo, np.float32).reshape(D)
    wi_v = np.asarray(Wi, np.float32).reshape(D)
    wu1_v = np.asarray(Wu1, np.float32).reshape(D)

    # host-side tiny projections: scores = raw @ (q@Wk * inv_sqrt_d).T
    qWk = (questions @ Wk) * inv_sqrt_d          # [Q, D]
    qWo = (questions @ Wo) * inv_sqrt_q
    qwk = to_f8(np.concatenate(
        [qWk.T * QW_SC, (-QW_SC) * wi_v[:, None]], axis=1))   # [D, Q1]
    qwo = to_f8(np.concatenate(
        [qWo.T * QW_SC, (-QW_SC) * wu1_v[:, None]], axis=1))
    bq1 = np.concatenate(
        [(questions @ bk_v) * inv_sqrt_d,
         [-bi_v]]).astype(np.float32).reshape(Q1, 1)
    bq2 = np.concatenate(
        [(questions @ bo_v) * inv_sqrt_q,
         [-cgate_v]]).astype(np.float32).reshape(Q1, 1)
    wu2B = np.ascontiguousarray(
        np.broadcast_to(np.asarray(Wu2, np.float32).reshape(1, D), (Q, D)))

    in_maps = []
    for r in range(NCORES):
        bs = slice(r * BL, (r + 1) * BL)
        rawT = to_f8(np.ascontiguousarray(
            raw[bs].transpose(0, 2, 1)) * RAW_SC)
        rawN = to_f8(raw[bs] * RAW_SC)
        pdT = to_f8(np.ascontiguousarray(
            post_dec[bs].transpose(0, 2, 1)) * RAW_SC)
        pdN = np.ascontiguousarray(post_dec[bs]).astype(BF)
        in_maps.append({
            "rawT": rawT, "rawN": rawN, "pdT": pdT, "pdN": pdN,
            "qwk": qwk, "qwo": qwo, "bq1": bq1, "bq2": bq2, "wu2B": wu2B,
        })

    res = run_bass_kernel_spmd(nc, in_maps, core_ids=list(range(NCORES)),
                               trace=_trace)
    out = np.concatenate([res.results[r]["out"] for r in range(NCORES)],
                         axis=0)
    if _trace:
        kernel._last_result = res
    return out


# revision 28
# speedup vs baseline: 1.1058x; 1.1058x over previous
"""Trainium2 Bass kernel for nn_Pndb_43344809951805 (scatter_memory).

Data-parallel over batch B=16 across 8 NeuronCores (2 batches/core).

Key algebraic restructure: scores = q@(raw@Wk.T).T = (q@Wk)@raw.T, so the
[D,D] k/ko GEMMs are replaced by tiny host-side [Q,D] projections (qWk, qWo).
The v-gate (raw.Wi) and u-gate (pd.Wu1) dot products ride along as a 65th
output row of the scores/s2 matmuls with sign-flipped weights, so one Exp
activation yields both exp(scores) and exp(-gate_arg). Phase 1 recovers the
gate row via the 65-wide PE transposes; phase 2 recovers it via a third
column of the Z2/G2 ones|aw matmul. All big matmuls are fp8e4 DoubleRow
(K=256/instruction). A = mean over B needs an AllReduce of the per-core
partial [Q,D] A (plus the A.Wu2 row), overlapped with phase-2 partA work.
"""
import sys

sys.path.insert(0, "/opt/trn_rl_repo")

import numpy as np
import ml_dtypes

import concourse.bass as bass
import concourse.bacc as bacc
import concourse.mybir as mybir
import concourse.tile as tile
from concourse import masks
from concourse.bass_utils import run_bass_kernel_spmd

F32 = mybir.dt.float32
BF16 = mybir.dt.bfloat16
F8 = mybir.dt.float8e4
AF = mybir.ActivationFunctionType
ALU = mybir.AluOpType
DR = mybir.MatmulPerfMode.DoubleRow
BF = ml_dtypes.bfloat16
F8NP = mybir.dt.np(mybir.dt.float8e4)
RAW_SC = 16.0    # fp8 scale on raw/post_dec activations
QW_SC = 16.0     # fp8 scale on qWk/qWo/-Wi/-Wu1 stationary weights
SC_SC = RAW_SC * QW_SC          # psum score scale (256)
INV_SC = 1.0 / SC_SC

B, S, D, Q = 16, 2048, 1024, 64
Q1 = Q + 1                # extra gate row
NCORES = 8
BL = B // NCORES          # local batches per core
SBLK = 512                # s-block (matmul moving free dim)
NSB = S // SBLK           # 4 s-blocks per batch
NCH = S // 128            # 16 s-chunks per batch
NJ = D // 128             # 8 contraction chunks
CPB = SBLK // 128         # 4 chunks per s-block

_prog_cache = {}


def _build(stage: str = "full"):
    nc = bacc.Bacc("TRN2", target_bir_lowering=False, debug=False,
                   enable_asserts=False, num_devices=NCORES)

    rawT_d = nc.dram_tensor("rawT", [BL, D, S], F8, kind="ExternalInput")
    rawN_d = nc.dram_tensor("rawN", [BL, S, D], F8, kind="ExternalInput")
    pdT_d = nc.dram_tensor("pdT", [BL, D, S], F8, kind="ExternalInput")
    pdN_d = nc.dram_tensor("pdN", [BL, S, D], BF16, kind="ExternalInput")
    qwk_d = nc.dram_tensor("qwk", [D, Q1], F8, kind="ExternalInput")
    qwo_d = nc.dram_tensor("qwo", [D, Q1], F8, kind="ExternalInput")
    bq1_d = nc.dram_tensor("bq1", [Q1, 1], F32, kind="ExternalInput")
    bq2_d = nc.dram_tensor("bq2", [Q1, 1], F32, kind="ExternalInput")
    wu2B_d = nc.dram_tensor("wu2B", [Q, D], F32, kind="ExternalInput")
    out_d = nc.dram_tensor("out", [BL, S, D], F32, kind="ExternalOutput")

    # [D, X] -> [128, NJ, X] chunked view for single-descriptor DMA
    def chunked(ap):
        return ap.rearrange("(j p) x -> p j x", p=128)

    with tile.TileContext(nc) as tc:
        with (
            tc.tile_pool(name="const", bufs=1) as cp,
            tc.tile_pool(name="dram", bufs=1, space="DRAM") as dram,
        ):
            ident = cp.tile([128, 128], BF16, tag="ident")
            masks.make_identity(nc, ident[:])

            # per-chunk width padded to 80: DoubleRow LDWEIGHTS requires the
            # inter-subtile step to be a multiple of 16
            QP = 80
            qwk8 = cp.tile([128, NJ, QP], F8, tag="qwk8")
            qwo8 = cp.tile([128, NJ, QP], F8, tag="qwo8")
            bq1 = cp.tile([Q1, 1], F32, tag="bq1")
            bq2 = cp.tile([Q1, 1], F32, tag="bq2")
            wu2B = cp.tile([Q, D], F32, tag="wu2B")

            # critical-path weights first
            nc.sync.dma_start(qwk8[:, :, 0:Q1], chunked(qwk_d[:, :]))
            nc.sync.dma_start(bq1[:], bq1_d[:, :])

            A_acc = cp.tile([Q, D], F32, tag="A_acc")
            A_f32 = cp.tile([Q, D], F32, tag="A_f32")
            A_bf = cp.tile([Q, D], BF16, tag="A_bf")
            # zg rhs: [ones | aw | e_gate] ([Q1, 3]); aw lands after the AR
            awo = cp.tile([Q1, 3], BF16, tag="awo")
            nc.vector.memset(awo[:], 0.0)
            nc.vector.memset(awo[0:Q, 0:1], 1.0)
            nc.vector.memset(awo[Q:Q1, 2:3], 1.0)
            scrA = cp.tile([Q, D], F32, tag="scrA")
            aw = cp.tile([Q, 1], F32, tag="aw")
            ar_in = dram.tile([Q + 1, D + 8], BF16)
            ar_out = dram.tile([Q + 1, D + 8], BF16)
            awz = cp.tile([Q, 8], BF16, tag="awz")
            nc.vector.memset(awz[:], 0.0)
            zrow = cp.tile([1, D + 8], BF16, tag="zrow")
            nc.vector.memset(zrow[:], 0.0)
            nc.gpsimd.dma_start(ar_in[0:Q, D:D + 8], awz[:])
            nc.gpsimd.dma_start(ar_in[Q:Q + 1, Q:D + 8], zrow[:, Q:D + 8])

            # ================= PHASE 1 =================
            with (
                tc.tile_pool(name="p1", bufs=1) as p1,
                tc.tile_pool(name="p1ps", bufs=1, space="PSUM") as p1ps,
            ):
                PFP = 2  # prefetch depth in 2-sblock pairs

                def load_raw2(pair):
                    b, sb2 = divmod(pair, NSB // 2)
                    t = p1.tile([128, NJ, 2 * SBLK], F8, tag="raw8",
                                name="raw8", bufs=PFP + 1)
                    nc.sync.dma_start(
                        t[:], chunked(rawT_d[b])[
                            :, :, sb2 * 2 * SBLK:(sb2 + 1) * 2 * SBLK])
                    return t

                def load_rns(idx):
                    b, sb = all_p1[idx]
                    t = p1.tile([128, CPB, D], F8, tag="rns",
                                name="rns", bufs=2 * PFP + 2)
                    nc.sync.dma_start(
                        t[:],
                        rawN_d[b, sb * SBLK:(sb + 1) * SBLK, :].rearrange(
                            "(c p) d -> p c d", p=128))
                    return t

                all_p1 = [(b, sb) for b in range(BL) for sb in range(NSB)]
                NP1 = len(all_p1)
                raw_fifo = [load_raw2(k) for k in range(PFP)]
                rns_fifo = [load_rns(k) for k in range(2 * PFP)]
                nc.sync.dma_start(qwo8[:, :, 0:Q1], chunked(qwo_d[:, :]))
                nc.sync.dma_start(bq2[:], bq2_d[:, :])
                nc.sync.dma_start(wu2B[:], wu2B_d[:])

                # 3-stage software pipeline: front(i) = scores+exp,
                # mid(i-1) = transposes+gate+uts, backA(i-2) = A-matmuls.
                # Keeps the tensor queue gapless (no head-of-line waits on
                # the scalar/vector gate chain).
                zp_of = {}
                aps_of = {}

                def front(idx):
                    b, sb = all_p1[idx]
                    if idx % 2 == 0:
                        if idx // 2 + PFP < NP1 // 2:
                            raw_fifo.append(load_raw2(idx // 2 + PFP))
                        front.raw2 = raw_fifo.pop(0)
                    rns_cur = rns_fifo.pop(0)
                    if idx + 2 * PFP < NP1:
                        rns_fifo.append(load_rns(idx + 2 * PFP))
                    if sb == 0:
                        zp_of[b] = p1.tile([Q1, NSB], F32, tag="Zp", bufs=2,
                                           name="Zp")
                    ssl = slice((idx % 2) * SBLK, (idx % 2 + 1) * SBLK)
                    # scores (+gate row 64) = qwk8.T @ raw8, fp8 DR
                    sc_ps = p1ps.tile([Q1, SBLK], F32, tag="sc_ps", bufs=2)
                    for j0 in range(NJ // 2):
                        nc.tensor.matmul(
                            sc_ps[:],
                            qwk8[:, 2 * j0:2 * j0 + 2, 0:Q1],
                            front.raw2[:, 2 * j0:2 * j0 + 2, ssl],
                            start=(j0 == 0), stop=(j0 == NJ // 2 - 1),
                            perf_mode=DR)
                    # U rows 0..63 = exp(scores); row 64 = exp(-raw.Wi-bi)
                    U = p1.tile([Q1, SBLK], BF16, tag="U", bufs=3)
                    nc.scalar.activation(U[:], sc_ps[:], AF.Exp,
                                         scale=INV_SC, bias=bq1[:],
                                         accum_out=zp_of[b][:, sb:sb + 1])
                    return (U, rns_cur, b, sb)

                def mid(st):
                    U, rns_cur, b, sb = st
                    # transpose (65-wide: gate-exp rides in column 64);
                    # per-chunk width padded to 66 so each bf16 slice stays
                    # 4-byte aligned in PSUM
                    ut_ps = p1ps.tile([128, CPB, Q1 + 1], BF16,
                                      tag="ut_ps", bufs=2)
                    for cc in range(CPB):
                        nc.tensor.transpose(
                            ut_ps[:, cc, 0:Q1],
                            U[:, cc * 128:(cc + 1) * 128],
                            ident[:Q1, :Q1])
                    # Gg = sigmoid(raw.Wi+bi) = 1/(1+exp(-)) per chunk
                    Gg = p1.tile([128, CPB], F32, tag="Gg", bufs=2)
                    nc.vector.tensor_scalar_add(
                        Gg[:], ut_ps[:, :, Q:Q + 1].squeeze(), 1.0)
                    nc.vector.reciprocal(Gg[:], Gg[:])
                    uts = p1.tile([128, CPB, Q], F8, tag="uts", bufs=3)
                    for cc in range(CPB):
                        nc.scalar.activation(uts[:, cc, :],
                                             ut_ps[:, cc, 0:Q], AF.Copy,
                                             scale=Gg[:, cc:cc + 1])
                    return (uts, rns_cur, b, sb)

                def back_a(st):
                    uts, rns_cur, b, sb = st
                    if sb == 0:
                        aps_of[b] = p1ps.tile([Q, D], F32, tag="A_ps",
                                              bufs=2, name="A_ps")
                    A_ps = aps_of[b]
                    # A += (U*g).T @ rawN, fp8 DR over chunk pairs
                    for c0 in range(CPB // 2):
                        cp2 = sb * (CPB // 2) + c0
                        for h in range(2):
                            hsl = slice(h * 512, (h + 1) * 512)
                            nc.tensor.matmul(
                                A_ps[:, hsl],
                                uts[:, 2 * c0:2 * c0 + 2, :],
                                rns_cur[:, 2 * c0:2 * c0 + 2, hsl],
                                start=(cp2 == 0),
                                stop=(cp2 == NSB * CPB // 2 - 1),
                                perf_mode=DR, skip_group_check=True)
                    if sb == NSB - 1:
                        # A_acc += A_ps / (16 * B * Z)  (16 = fp8 rawN scale)
                        Z1 = p1.tile([Q, 1], F32, tag="Z1", bufs=2)
                        nc.vector.tensor_reduce(Z1[:], zp_of[b][0:Q, :],
                                                mybir.AxisListType.X,
                                                ALU.add)
                        sA = p1.tile([Q, 1], F32, tag="sA", bufs=2)
                        nc.vector.reciprocal(sA[:], Z1[:])
                        nc.vector.tensor_scalar_mul(sA[:], sA[:],
                                                    1.0 / (B * RAW_SC))
                        if b == 0:
                            nc.vector.tensor_scalar_mul(A_acc[:], A_ps[:],
                                                        sA[:])
                        else:
                            nc.vector.scalar_tensor_tensor(
                                A_acc[:], A_ps[:], sA[:], A_acc[:],
                                ALU.mult, ALU.add)

                stages = []
                for idx in range(NP1):
                    stages.append(front(idx))
                    if len(stages) >= 2:
                        stages[-2] = mid(stages[-2])
                    if len(stages) >= 3:
                        back_a(stages.pop(0))
                stages[1] = mid(stages[1])
                back_a(stages.pop(0))
                back_a(stages.pop(0))

                # aw_local = A_acc . Wu2 rides along in the AllReduce
                nc.vector.scalar_tensor_tensor(
                    scrA[:], A_acc[:], 1.0, wu2B[:],
                    ALU.mult, ALU.mult, accum_out=aw[:])
                nc.gpsimd.dma_start(ar_in[0:Q, 0:D], A_acc[:])
                nc.gpsimd.dma_start(
                    ar_in[Q:Q + 1, 0:Q].rearrange("a b -> b a"), aw[:])

            # ---- AllReduce of partial A across the 8 cores ----
            if stage == "p1":
                nc.sync.dma_start(out_d[0, 0:Q, :], A_acc[:])
            else:
                if stage == "p2":
                    arr = ar_in
                else:
                    nc.gpsimd.collective_compute(
                        "AllReduce", ALU.add,
                        replica_groups=[list(range(NCORES))],
                        ins=[ar_in.opt()], outs=[ar_out.opt()],
                    )
                    arr = ar_out
                if stage == "ar":
                    nc.gpsimd.dma_start(A_f32[:], arr[0:Q, 0:D])
                nc.gpsimd.dma_start(A_bf[:], arr[0:Q, 0:D])
                nc.gpsimd.dma_start(
                    awo[0:Q, 1:2], arr[Q:Q + 1, 0:Q].rearrange("a b -> b a"))

            # ================= PHASE 2 =================
            if stage == "p1":
                pass  # skip phase 2
            else:
              with (
                  tc.tile_pool(name="p2", bufs=1) as p2,
                  tc.tile_pool(name="p2ps", bufs=1, space="PSUM") as p2ps,
              ):
                  NP2 = BL * NSB + 1

                  pdt_of = {}

                  def part_a(b, sb):
                      ssl = slice(sb * SBLK, (sb + 1) * SBLK)
                      if sb % 2 == 0:
                          t = p2.tile([128, NJ, 2 * SBLK], F8, tag="pdt8",
                                      name="pdt8", bufs=2)
                          nc.sync.dma_start(
                              t[:], chunked(pdT_d[b])[
                                  :, :, sb * SBLK:(sb + 2) * SBLK])
                          pdt_of[0] = t
                      pdt8 = pdt_of[0][:, :, (sb % 2) * SBLK:
                                       (sb % 2 + 1) * SBLK]
                      pdn = p2.tile([128, CPB, D], BF16, tag="pdn",
                                    name="pdn", bufs=NP2)
                      nc.sync.dma_start(
                          pdn[:],
                          pdN_d[b, ssl, :].rearrange("(c p) d -> p c d",
                                                     p=128))
                      # s2 (+gate row 64) = qwo8.T @ pdt8, fp8 DR
                      sc2_ps = p2ps.tile([Q1, SBLK], F32, tag="s2_ps",
                                         bufs=2)
                      for j0 in range(NJ // 2):
                          nc.tensor.matmul(
                              sc2_ps[:],
                              qwo8[:, 2 * j0:2 * j0 + 2, 0:Q1],
                              pdt8[:, 2 * j0:2 * j0 + 2, :],
                              start=(j0 == 0), stop=(j0 == NJ // 2 - 1),
                              perf_mode=DR)
                      # U2 rows 0..63 = exp(s2); row 64 = exp(-pd.Wu1-cg)
                      U2 = p2.tile([Q1, SBLK], BF16, tag="U2", bufs=NP2)
                      nc.scalar.activation(U2[:], sc2_ps[:], AF.Exp,
                                           scale=INV_SC, bias=bq2[:])
                      return (U2, pdn, b, sb)

                  def part_b_z(st):
                      (U2, pdn, b, sb) = st
                      # per chunk: [Z2 | G2 | E1] = U2[q,s].T @ [ones|aw|e64]
                      zg_ps = p2ps.tile([128, CPB, 3], F32, tag="zg",
                                        bufs=2)
                      for cc in range(CPB):
                          nc.tensor.matmul(
                              zg_ps[:, cc, :],
                              U2[:, cc * 128:(cc + 1) * 128], awo[:],
                              start=(cc == 0), stop=(cc == CPB - 1),
                              skip_group_check=True)
                      # SC = sigmoid(G1+G2/Z2+cg)/Z2
                      #    = 1/((1+E1*exp(-G2/Z2))*Z2)
                      rz = p2.tile([128, CPB], F32, tag="rz", bufs=2)
                      nc.vector.reciprocal(rz[:],
                                           zg_ps[:, :, 0:1].squeeze())
                      t4 = p2.tile([128, CPB], F32, tag="t4", bufs=2)
                      nc.vector.tensor_mul(t4[:], zg_ps[:, :, 1:2].squeeze(),
                                           rz[:])
                      e4 = p2.tile([128, CPB], F32, tag="e4", bufs=2)
                      nc.scalar.activation(e4[:], t4[:], AF.Exp, scale=-1.0)
                      nc.vector.tensor_mul(e4[:], e4[:],
                                           zg_ps[:, :, 2:3].squeeze())
                      nc.vector.tensor_scalar_add(e4[:], e4[:], 1.0)
                      nc.vector.reciprocal(e4[:], e4[:])
                      SC = p2.tile([128, CPB], F32, tag="SC", bufs=2)
                      nc.vector.tensor_mul(SC[:], e4[:], rz[:])
                      return (U2, pdn, SC, b, sb)

                  def part_b_out(st):
                      (U2, pdn, SC, b, sb) = st
                      # A2 + fused residual; h0 on DVE, h1 via scalar+gpsimd
                      for cc in range(CPB):
                          c = sb * CPB + cc
                          outt = p2.tile([128, D], F32, tag="outt", bufs=3)
                          for h in range(2):
                              hsl = slice(h * 512, (h + 1) * 512)
                              a2_ps = p2ps.tile([128, 512], F32,
                                                tag="a2_ps", bufs=4)
                              nc.tensor.matmul(
                                  a2_ps[:], U2[0:Q, cc * 128:(cc + 1) * 128],
                                  A_bf[:, hsl], start=True, stop=True)
                              nc.vector.scalar_tensor_tensor(
                                  outt[:, hsl], a2_ps[:],
                                  SC[:, cc:cc + 1],
                                  pdn[:, cc, hsl], ALU.mult, ALU.add)
                          nc.gpsimd.dma_start(
                              out_d[b, c * 128:(c + 1) * 128, :], outt[:])

                  all_sb = [(b, sb) for b in range(BL) for sb in range(NSB)]
                  pending = [part_a(b, sb) for b, sb in all_sb]
                  zstates = []
                  for st in pending:
                      z = part_b_z(st)
                      if zstates:
                          part_b_out(zstates.pop(0))
                      zstates.append(z)
                  for z in zstates:
                      part_b_out(z)

            if stage == "ar":
                nc.sync.dma_start(out_d[0, 0:Q, :], A_f32[:])
    nc.compile()
    return nc


def _get_prog():
    if "p" not in _prog_cache:
        _prog_cache["p"] = _build()
    return _prog_cache["p"]


def kernel(raw, post_dec, mask, questions, Wk, bk, Wi, bi, Wo, bo,
           Wu1, bu1, Wu2, bu2, b1, _trace=False):
    raw = np.asarray(raw, dtype=np.float32)
    post_dec = np.asarray(post_dec, dtype=np.float32)
    questions = np.asarray(questions, dtype=np.float32)
    Wk = np.asarray(Wk, dtype=np.float32)
    Wo = np.asarray(Wo, dtype=np.float32)

    bi_v = float(np.asarray(bi).reshape(-1)[0])
    cgate_v = float(np.asarray(bu1).reshape(-1)[0]
                    + np.asarray(bu2).reshape(-1)[0]
                    + np.asarray(b1).reshape(-1)[0])
    nc = _get_prog()

    inv_sqrt_d = np.float32(1.0 / np.sqrt(D))
    inv_sqrt_q = np.float32(1.0 / np.sqrt(Q))

    def to_f8(x):
        return np.clip(x, -240.0, 240.0).astype(F8NP)

    bk_v = np.asarray(bk, np.float32).reshape(D)
    bo_v = np.asarray(bo, np.float32).reshape(D)
    wi_v = np.asarray(Wi, np.float32).reshape(D)
    wu1_v = np.asarray(Wu1, np.float32).reshape(D)

    # host-side tiny projections: scores = raw @ (q@Wk * inv_sqrt_d).T
    qWk = (questions @ Wk) * inv_sqrt_d          # [Q, D]
    qWo = (questions @ Wo) * inv_sqrt_q
    qwk = to_f8(np.concatenate(
        [qWk.T * QW_SC, (-QW_SC) * wi_v[:, None]], axis=1))   # [D, Q1]
    qwo = to_f8(np.concatenate(
        [qWo.T * QW_SC, (-QW_SC) * wu1_v[:, None]], axis=1))
    bq1 = np.concatenate(
        [(questions @ bk_v) * inv_sqrt_d,
         [-bi_v]]).astype(np.float32).reshape(Q1, 1)
    bq2 = np.concatenate(
        [(questions @ bo_v) * inv_sqrt_q,
         [-cgate_v]]).astype(np.float32).reshape(Q1, 1)
    wu2B = np.ascontiguousarray(
        np.broadcast_to(np.asarray(Wu2, np.float32).reshape(1, D), (Q, D)))

    in_maps = []
    for r in range(NCORES):
        bs = slice(r * BL, (r + 1) * BL)
        rawT = to_f8(np.ascontiguousarray(
            raw[bs].transpose(0, 2, 1)) * RAW_SC)
        rawN = to_f8(raw[bs] * RAW_SC)
        pdT = to_f8(np.ascontiguousarray(
            post_dec[bs].transpose(0, 2, 1)) * RAW_SC)
        pdN = np.ascontiguousarray(post_dec[bs]).astype(BF)
        in_maps.append({
            "rawT": rawT, "rawN": rawN, "pdT": pdT, "pdN": pdN,
            "qwk": qwk, "qwo": qwo, "bq1": bq1, "bq2": bq2, "wu2B": wu2B,
        })

    res = run_bass_kernel_spmd(nc, in_maps, core_ids=list(range(NCORES)),
                               trace=_trace)
    out = np.concatenate([res.results[r]["out"] for r in range(NCORES)],
                         axis=0)
    if _trace:
        kernel._last_result = res
    return out


# revision 43
# speedup vs baseline: 1.1614x; 1.0503x over previous
"""Trainium2 Bass kernel for nn_Pndb_43344809951805 (scatter_memory).

Data-parallel over batch B=16 across 8 NeuronCores (2 batches/core).

Key algebraic restructure: scores = q@(raw@Wk.T).T = (q@Wk)@raw.T, so the
[D,D] k/ko GEMMs are replaced by tiny host-side [Q,D] projections (qWk, qWo).
The v-gate (raw.Wi) and u-gate (pd.Wu1) dot products ride along as a 65th
output row of the scores/s2 matmuls with sign-flipped weights, so one Exp
activation yields both exp(scores) and exp(-gate_arg). Phase 1 recovers the
gate row via the 65-wide PE transposes; phase 2 recovers it via a third
column of the Z2/G2 ones|aw matmul. All big matmuls are fp8e4 DoubleRow
(K=256/instruction). A = mean over B needs an AllReduce of the per-core
partial [Q,D] A (plus the A.Wu2 row), overlapped with phase-2 partA work.
"""
import sys

sys.path.insert(0, "/opt/trn_rl_repo")

import numpy as np
import ml_dtypes

import concourse.bass as bass
import concourse.bacc as bacc
import concourse.mybir as mybir
import concourse.tile as tile
from concourse import masks
from concourse.bass_utils import run_bass_kernel_spmd

F32 = mybir.dt.float32
BF16 = mybir.dt.bfloat16
F8 = mybir.dt.float8e4
AF = mybir.ActivationFunctionType
ALU = mybir.AluOpType
DR = mybir.MatmulPerfMode.DoubleRow
BF = ml_dtypes.bfloat16
F8NP = mybir.dt.np(mybir.dt.float8e4)
RAW_SC = 16.0    # fp8 scale on raw/post_dec activations
QW_SC = 16.0     # fp8 scale on qWk/qWo/-Wi/-Wu1 stationary weights
SC_SC = RAW_SC * QW_SC          # psum score scale (256)
INV_SC = 1.0 / SC_SC
AR_SC = 32.0     # fp8 scale on the AllReduce payload (A, aw)

B, S, D, Q = 16, 2048, 1024, 64
Q1 = Q + 1                # extra gate row
NCORES = 8
BL = B // NCORES          # local batches per core
SBLK = 512                # s-block (matmul moving free dim)
NSB = S // SBLK           # 4 s-blocks per batch
NCH = S // 128            # 16 s-chunks per batch
NJ = D // 128             # 8 contraction chunks
CPB = SBLK // 128         # 4 chunks per s-block

_prog_cache = {}


def _build(stage: str = "full"):
    nc = bacc.Bacc("TRN2", target_bir_lowering=False, debug=False,
                   enable_asserts=False, num_devices=NCORES)

    rawT_d = nc.dram_tensor("rawT", [BL, D, S], F8, kind="ExternalInput")
    rawN_d = nc.dram_tensor("rawN", [BL, S, D], F8, kind="ExternalInput")
    pdT_d = nc.dram_tensor("pdT", [BL, D, S], F8, kind="ExternalInput")
    pdN_d = nc.dram_tensor("pdN", [BL, S, D], BF16, kind="ExternalInput")
    qwk_d = nc.dram_tensor("qwk", [D, Q1], F8, kind="ExternalInput")
    qwo_d = nc.dram_tensor("qwo", [D, Q1], F8, kind="ExternalInput")
    bq1_d = nc.dram_tensor("bq1", [Q1, 1], F32, kind="ExternalInput")
    bq2_d = nc.dram_tensor("bq2", [Q1, 1], F32, kind="ExternalInput")
    wu2B_d = nc.dram_tensor("wu2B", [Q, D], F32, kind="ExternalInput")
    out_d = nc.dram_tensor("out", [BL, S, D], F32, kind="ExternalOutput")

    # [D, X] -> [128, NJ, X] chunked view for single-descriptor DMA
    def chunked(ap):
        return ap.rearrange("(j p) x -> p j x", p=128)

    with tile.TileContext(nc) as tc:
        with (
            tc.tile_pool(name="const", bufs=1) as cp,
            tc.tile_pool(name="dram", bufs=1, space="DRAM") as dram,
        ):
            ident = cp.tile([128, 128], BF16, tag="ident")
            masks.make_identity(nc, ident[:])

            # per-chunk width padded to 80: DoubleRow LDWEIGHTS requires the
            # inter-subtile step to be a multiple of 16
            QP = 80
            qwk8 = cp.tile([128, NJ, QP], F8, tag="qwk8")
            qwo8 = cp.tile([128, NJ, QP], F8, tag="qwo8")
            bq1 = cp.tile([Q1, 1], F32, tag="bq1")
            bq2 = cp.tile([Q1, 1], F32, tag="bq2")
            wu2B = cp.tile([Q, D], F32, tag="wu2B")

            # critical-path weights first
            nc.sync.dma_start(qwk8[:, :, 0:Q1], chunked(qwk_d[:, :]))
            nc.sync.dma_start(bq1[:], bq1_d[:, :])

            A_acc = cp.tile([Q, D], F32, tag="A_acc")
            A_f32 = cp.tile([Q, D], F32, tag="A_f32")
            A_bf = cp.tile([Q, D], BF16, tag="A_bf")
            # zg rhs: [ones | aw | e_gate] ([Q1, 3]); aw lands after the AR
            awo = cp.tile([Q1, 3], BF16, tag="awo")
            nc.vector.memset(awo[:], 0.0)
            nc.vector.memset(awo[0:Q, 0:1], 1.0)
            nc.vector.memset(awo[Q:Q1, 2:3], 1.0)
            scrA = cp.tile([Q, D], F32, tag="scrA")
            aw = cp.tile([Q, 1], F32, tag="aw")
            ar_in = dram.tile([Q + 1, D + 8], F8)
            ar_out = dram.tile([Q + 1, D + 8], F8)
            awz = cp.tile([Q, 8], F8, tag="awz")
            nc.vector.memset(awz[:], 0.0)
            zrow = cp.tile([1, D + 8], F8, tag="zrow")
            nc.vector.memset(zrow[:], 0.0)
            nc.gpsimd.dma_start(ar_in[0:Q, D:D + 8], awz[:])
            nc.gpsimd.dma_start(ar_in[Q:Q + 1, Q:D + 8], zrow[:, Q:D + 8])

            # ================= PHASE 1 =================
            with (
                tc.tile_pool(name="p1", bufs=1) as p1,
                tc.tile_pool(name="p1ps", bufs=1, space="PSUM") as p1ps,
            ):
                PFP = 2  # prefetch depth in 2-sblock pairs

                def load_raw2(pair):
                    b, sb2 = divmod(pair, NSB // 2)
                    t = p1.tile([128, NJ, 2 * SBLK], F8, tag="raw8",
                                name="raw8", bufs=PFP + 1)
                    nc.sync.dma_start(
                        t[:], chunked(rawT_d[b])[
                            :, :, sb2 * 2 * SBLK:(sb2 + 1) * 2 * SBLK])
                    return t

                def load_rns(idx):
                    b, sb = all_p1[idx]
                    t = p1.tile([128, CPB, D], F8, tag="rns",
                                name="rns", bufs=2 * PFP + 2)
                    nc.sync.dma_start(
                        t[:],
                        rawN_d[b, sb * SBLK:(sb + 1) * SBLK, :].rearrange(
                            "(c p) d -> p c d", p=128))
                    return t

                all_p1 = [(b, sb) for b in range(BL) for sb in range(NSB)]
                NP1 = len(all_p1)
                raw_fifo = [load_raw2(k) for k in range(PFP)]
                rns_fifo = [load_rns(k) for k in range(2 * PFP)]
                nc.sync.dma_start(qwo8[:, :, 0:Q1], chunked(qwo_d[:, :]))
                nc.sync.dma_start(bq2[:], bq2_d[:, :])
                nc.sync.dma_start(wu2B[:], wu2B_d[:])

                # 3-stage software pipeline: front(i) = scores+exp,
                # mid(i-1) = transposes+gate+uts, backA(i-2) = A-matmuls.
                # Keeps the tensor queue gapless (no head-of-line waits on
                # the scalar/vector gate chain).
                zp_of = {}
                aps_of = {}

                def front(idx):
                    b, sb = all_p1[idx]
                    if idx % 2 == 0:
                        if idx // 2 + PFP < NP1 // 2:
                            raw_fifo.append(load_raw2(idx // 2 + PFP))
                        front.raw2 = raw_fifo.pop(0)
                    rns_cur = rns_fifo.pop(0)
                    if idx + 2 * PFP < NP1:
                        rns_fifo.append(load_rns(idx + 2 * PFP))
                    if sb == 0:
                        zp_of[b] = p1.tile([Q1, NSB], F32, tag="Zp", bufs=2,
                                           name="Zp")
                    ssl = slice((idx % 2) * SBLK, (idx % 2 + 1) * SBLK)
                    # scores (+gate row 64) = qwk8.T @ raw8, fp8 DR
                    sc_ps = p1ps.tile([Q1, SBLK], F32, tag="sc_ps", bufs=2)
                    for j0 in range(NJ // 2):
                        nc.tensor.matmul(
                            sc_ps[:],
                            qwk8[:, 2 * j0:2 * j0 + 2, 0:Q1],
                            front.raw2[:, 2 * j0:2 * j0 + 2, ssl],
                            start=(j0 == 0), stop=(j0 == NJ // 2 - 1),
                            perf_mode=DR)
                    # U rows 0..63 = exp(scores); row 64 = exp(-raw.Wi-bi)
                    U = p1.tile([Q1, SBLK], BF16, tag="U", bufs=3)
                    nc.scalar.activation(U[:], sc_ps[:], AF.Exp,
                                         scale=INV_SC, bias=bq1[:],
                                         accum_out=zp_of[b][:, sb:sb + 1])
                    return (U, rns_cur, b, sb)

                def mid(st):
                    U, rns_cur, b, sb = st
                    # transpose (65-wide: gate-exp rides in column 64);
                    # per-chunk width padded to 66 so each bf16 slice stays
                    # 4-byte aligned in PSUM
                    ut_ps = p1ps.tile([128, CPB, Q1 + 1], BF16,
                                      tag="ut_ps", bufs=2)
                    for cc in range(CPB):
                        nc.tensor.transpose(
                            ut_ps[:, cc, 0:Q1],
                            U[:, cc * 128:(cc + 1) * 128],
                            ident[:Q1, :Q1])
                    # Gg = sigmoid(raw.Wi+bi) = 1/(1+exp(-)) per chunk
                    Gg = p1.tile([128, CPB], F32, tag="Gg", bufs=2)
                    nc.vector.tensor_scalar_add(
                        Gg[:], ut_ps[:, :, Q:Q + 1].squeeze(), 1.0)
                    nc.vector.reciprocal(Gg[:], Gg[:])
                    uts = p1.tile([128, CPB, Q], F8, tag="uts", bufs=3)
                    for cc in range(CPB):
                        nc.scalar.activation(uts[:, cc, :],
                                             ut_ps[:, cc, 0:Q], AF.Copy,
                                             scale=Gg[:, cc:cc + 1])
                    return (uts, rns_cur, b, sb)

                def back_a(st):
                    uts, rns_cur, b, sb = st
                    if sb == 0:
                        aps_of[b] = p1ps.tile([Q, D], F32, tag="A_ps",
                                              bufs=2, name="A_ps")
                    A_ps = aps_of[b]
                    # A += (U*g).T @ rawN, fp8 DR over chunk pairs
                    for c0 in range(CPB // 2):
                        cp2 = sb * (CPB // 2) + c0
                        for h in range(2):
                            hsl = slice(h * 512, (h + 1) * 512)
                            nc.tensor.matmul(
                                A_ps[:, hsl],
                                uts[:, 2 * c0:2 * c0 + 2, :],
                                rns_cur[:, 2 * c0:2 * c0 + 2, hsl],
                                start=(cp2 == 0),
                                stop=(cp2 == NSB * CPB // 2 - 1),
                                perf_mode=DR, skip_group_check=True)
                    if sb == NSB - 1:
                        # A_acc += A_ps / (16 * B * Z)  (16 = fp8 rawN scale)
                        Z1 = p1.tile([Q, 1], F32, tag="Z1", bufs=2)
                        nc.vector.tensor_reduce(Z1[:], zp_of[b][0:Q, :],
                                                mybir.AxisListType.X,
                                                ALU.add)
                        sA = p1.tile([Q, 1], F32, tag="sA", bufs=2)
                        nc.vector.reciprocal(sA[:], Z1[:])
                        nc.vector.tensor_scalar_mul(
                            sA[:], sA[:], AR_SC / (B * RAW_SC))
                        if b == 0:
                            nc.vector.tensor_scalar_mul(A_acc[:], A_ps[:],
                                                        sA[:])
                        else:
                            nc.vector.scalar_tensor_tensor(
                                A_acc[:], A_ps[:], sA[:], A_acc[:],
                                ALU.mult, ALU.add)

                stages = []
                for idx in range(NP1):
                    stages.append(front(idx))
                    if len(stages) >= 2:
                        stages[-2] = mid(stages[-2])
                    if len(stages) >= 3:
                        back_a(stages.pop(0))
                stages[1] = mid(stages[1])
                back_a(stages.pop(0))
                back_a(stages.pop(0))

                # aw_local = A_acc . Wu2 rides along in the AllReduce
                nc.vector.scalar_tensor_tensor(
                    scrA[:], A_acc[:], 1.0, wu2B[:],
                    ALU.mult, ALU.mult, accum_out=aw[:])
                nc.gpsimd.dma_start(ar_in[0:Q, 0:D], A_acc[:])
                nc.gpsimd.dma_start(
                    ar_in[Q:Q + 1, 0:Q].rearrange("a b -> b a"), aw[:])

            # ---- AllReduce of partial A across the 8 cores ----
            if stage == "p1":
                nc.sync.dma_start(out_d[0, 0:Q, :], A_acc[:])
            else:
                if stage == "p2":
                    arr = ar_in
                else:
                    nc.gpsimd.collective_compute(
                        "AllReduce", ALU.add,
                        replica_groups=[list(range(NCORES))],
                        ins=[ar_in.opt()], outs=[ar_out.opt()],
                    )
                    arr = ar_out
                if stage == "ar":
                    nc.gpsimd.dma_start(A_f32[:], arr[0:Q, 0:D])
                nc.gpsimd.dma_start(A_bf[:], arr[0:Q, 0:D])
                nc.gpsimd.dma_start(
                    awo[0:Q, 1:2], arr[Q:Q + 1, 0:Q].rearrange("a b -> b a"))

            # ================= PHASE 2 =================
            if stage == "p1":
                pass  # skip phase 2
            else:
              with (
                  tc.tile_pool(name="p2", bufs=1) as p2,
                  tc.tile_pool(name="p2ps", bufs=1, space="PSUM") as p2ps,
              ):
                  NP2 = BL * NSB + 1

                  pdt_of = {}

                  def part_a(b, sb):
                      ssl = slice(sb * SBLK, (sb + 1) * SBLK)
                      if sb % 2 == 0:
                          t = p2.tile([128, NJ, 2 * SBLK], F8, tag="pdt8",
                                      name="pdt8", bufs=2)
                          nc.sync.dma_start(
                              t[:], chunked(pdT_d[b])[
                                  :, :, sb * SBLK:(sb + 2) * SBLK])
                          pdt_of[0] = t
                      pdt8 = pdt_of[0][:, :, (sb % 2) * SBLK:
                                       (sb % 2 + 1) * SBLK]
                      pdn = p2.tile([128, CPB, D], BF16, tag="pdn",
                                    name="pdn", bufs=NP2)
                      nc.sync.dma_start(
                          pdn[:],
                          pdN_d[b, ssl, :].rearrange("(c p) d -> p c d",
                                                     p=128))
                      # s2 (+gate row 64) = qwo8.T @ pdt8, fp8 DR
                      sc2_ps = p2ps.tile([Q1, SBLK], F32, tag="s2_ps",
                                         bufs=2)
                      for j0 in range(NJ // 2):
                          nc.tensor.matmul(
                              sc2_ps[:],
                              qwo8[:, 2 * j0:2 * j0 + 2, 0:Q1],
                              pdt8[:, 2 * j0:2 * j0 + 2, :],
                              start=(j0 == 0), stop=(j0 == NJ // 2 - 1),
                              perf_mode=DR)
                      # U2 rows 0..63 = exp(s2); row 64 = exp(-pd.Wu1-cg)
                      U2 = p2.tile([Q1, SBLK], BF16, tag="U2", bufs=NP2)
                      nc.scalar.activation(U2[:], sc2_ps[:], AF.Exp,
                                           scale=INV_SC, bias=bq2[:])
                      return (U2, pdn, b, sb)

                  def part_b_z(st):
                      (U2, pdn, b, sb) = st
                      # per chunk: [Z2 | G2 | E1] = U2[q,s].T @ [ones|aw|e64]
                      zg_ps = p2ps.tile([128, CPB, 3], F32, tag="zg",
                                        bufs=2)
                      for cc in range(CPB):
                          nc.tensor.matmul(
                              zg_ps[:, cc, :],
                              U2[:, cc * 128:(cc + 1) * 128], awo[:],
                              start=(cc == 0), stop=(cc == CPB - 1),
                              skip_group_check=True)
                      # SC = sigmoid(G1+G2/Z2+cg)/Z2
                      #    = 1/((1+E1*exp(-G2/Z2))*Z2)
                      rz = p2.tile([128, CPB], F32, tag="rz", bufs=2)
                      nc.vector.reciprocal(rz[:],
                                           zg_ps[:, :, 0:1].squeeze())
                      t4 = p2.tile([128, CPB], F32, tag="t4", bufs=2)
                      nc.vector.tensor_mul(t4[:], zg_ps[:, :, 1:2].squeeze(),
                                           rz[:])
                      e4 = p2.tile([128, CPB], F32, tag="e4", bufs=2)
                      nc.scalar.activation(e4[:], t4[:], AF.Exp,
                                           scale=-1.0 / AR_SC)
                      nc.vector.tensor_mul(e4[:], e4[:],
                                           zg_ps[:, :, 2:3].squeeze())
                      nc.vector.tensor_scalar_add(e4[:], e4[:], 1.0)
                      nc.vector.reciprocal(e4[:], e4[:])
                      SC = p2.tile([128, CPB], F32, tag="SC", bufs=2)
                      nc.vector.tensor_mul(SC[:], e4[:], rz[:])
                      nc.vector.tensor_scalar_mul(SC[:], SC[:], 1.0 / AR_SC)
                      return (U2, pdn, SC, b, sb)

                  def part_b_out(st):
                      (U2, pdn, SC, b, sb) = st
                      # A2 + fused residual; h0 on DVE, h1 via scalar+gpsimd
                      for cc in range(CPB):
                          c = sb * CPB + cc
                          outt = p2.tile([128, D], F32, tag="outt", bufs=3)
                          for h in range(2):
                              hsl = slice(h * 512, (h + 1) * 512)
                              a2_ps = p2ps.tile([128, 512], F32,
                                                tag="a2_ps", bufs=4)
                              nc.tensor.matmul(
                                  a2_ps[:], U2[0:Q, cc * 128:(cc + 1) * 128],
                                  A_bf[:, hsl], start=True, stop=True)
                              nc.vector.scalar_tensor_tensor(
                                  outt[:, hsl], a2_ps[:],
                                  SC[:, cc:cc + 1],
                                  pdn[:, cc, hsl], ALU.mult, ALU.add)
                          # alternate store queues so descriptor generation
                          # and write streams run in parallel
                          eng = nc.gpsimd if cc % 2 == 0 else nc.sync
                          eng.dma_start(
                              out_d[b, c * 128:(c + 1) * 128, :], outt[:])

                  all_sb = [(b, sb) for b in range(BL) for sb in range(NSB)]
                  pending = [part_a(b, sb) for b, sb in all_sb]
                  zstates = []
                  for st in pending:
                      z = part_b_z(st)
                      if zstates:
                          part_b_out(zstates.pop(0))
                      zstates.append(z)
                  for z in zstates:
                      part_b_out(z)

            if stage == "ar":
                nc.sync.dma_start(out_d[0, 0:Q, :], A_f32[:])
    nc.compile()
    return nc


def _get_prog():
    if "p" not in _prog_cache:
        _prog_cache["p"] = _build()
    return _prog_cache["p"]


def kernel(raw, post_dec, mask, questions, Wk, bk, Wi, bi, Wo, bo,
           Wu1, bu1, Wu2, bu2, b1, _trace=False):
    raw = np.asarray(raw, dtype=np.float32)
    post_dec = np.asarray(post_dec, dtype=np.float32)
    questions = np.asarray(questions, dtype=np.float32)
    Wk = np.asarray(Wk, dtype=np.float32)
    Wo = np.asarray(Wo, dtype=np.float32)

    bi_v = float(np.asarray(bi).reshape(-1)[0])
    cgate_v = float(np.asarray(bu1).reshape(-1)[0]
                    + np.asarray(bu2).reshape(-1)[0]
                    + np.asarray(b1).reshape(-1)[0])
    nc = _get_prog()

    inv_sqrt_d = np.float32(1.0 / np.sqrt(D))
    inv_sqrt_q = np.float32(1.0 / np.sqrt(Q))

    def to_f8(x):
        return np.clip(x, -240.0, 240.0).astype(F8NP)

    bk_v = np.asarray(bk, np.float32).reshape(D)
    bo_v = np.asarray(bo, np.float32).reshape(D)
    wi_v = np.asarray(Wi, np.float32).reshape(D)
    wu1_v = np.asarray(Wu1, np.float32).reshape(D)

    # host-side tiny projections: scores = raw @ (q@Wk * inv_sqrt_d).T
    qWk = (questions @ Wk) * inv_sqrt_d          # [Q, D]
    qWo = (questions @ Wo) * inv_sqrt_q
    qwk = to_f8(np.concatenate(
        [qWk.T * QW_SC, (-QW_SC) * wi_v[:, None]], axis=1))   # [D, Q1]
    qwo = to_f8(np.concatenate(
        [qWo.T * QW_SC, (-QW_SC) * wu1_v[:, None]], axis=1))
    bq1 = np.concatenate(
        [(questions @ bk_v) * inv_sqrt_d,
         [-bi_v]]).astype(np.float32).reshape(Q1, 1)
    bq2 = np.concatenate(
        [(questions @ bo_v) * inv_sqrt_q,
         [-cgate_v]]).astype(np.float32).reshape(Q1, 1)
    wu2B = np.ascontiguousarray(
        np.broadcast_to(np.asarray(Wu2, np.float32).reshape(1, D), (Q, D)))

    in_maps = []
    for r in range(NCORES):
        bs = slice(r * BL, (r + 1) * BL)
        rawT = to_f8(np.ascontiguousarray(
            raw[bs].transpose(0, 2, 1)) * RAW_SC)
        rawN = to_f8(raw[bs] * RAW_SC)
        pdT = to_f8(np.ascontiguousarray(
            post_dec[bs].transpose(0, 2, 1)) * RAW_SC)
        pdN = np.ascontiguousarray(post_dec[bs]).astype(BF)
        in_maps.append({
            "rawT": rawT, "rawN": rawN, "pdT": pdT, "pdN": pdN,
            "qwk": qwk, "qwo": qwo, "bq1": bq1, "bq2": bq2, "wu2B": wu2B,
        })

    res = run_bass_kernel_spmd(nc, in_maps, core_ids=list(range(NCORES)),
                               trace=_trace)
    out = np.concatenate([res.results[r]["out"] for r in range(NCORES)],
                         axis=0)
    if _trace:
        kernel._last_result = res
    return out
